# revision 10
# baseline (speedup 1.0000x reference)
"""Trainium2 Bass kernel for ContrastiveMaskedPatchSimilarity loss.

Computes: per-position cosine similarity along the channel axis of two
[32, 256, 64, 64] f32 tensors, then a masked mean -> scalar.

Strategy (pure data parallel over 8 NeuronCores, batch-sharded 4 each):
  - Layout on chip: [channel-chunk (128) = partitions, spatial (4096) = free].
    DMA of u/m tiles is contiguous per partition (16KB rows); mid-stream
    tiles are the full 2 MiB row block, with quarter tiles at the very
    start (fast ramp) and very end (short tail).
  - Elementwise products (u*m, u*u, m*m) on DVE/ACT, written as bf16.
  - Channel reduction via TensorE: product slice [128ch x 128pos] is the
    *stationary* operand (lhsT), rhs = ones[128,1] bf16 -> out[128pos, 1]
    lands position-major in PSUM. The two channel chunks accumulate into
    the same PSUM column (start=ch==0, stop=ch==1).
  - Per batch: one [128, 3*NPB] PSUM->SBUF copy, DMA'd straight to DRAM.
    The cosine division, mask, and final mean run on the host (tiny).
"""

import sys
from contextlib import ExitStack

import numpy as np

sys.path.insert(0, "/opt/trn_rl_repo")

import ml_dtypes  # noqa: E402

import concourse.bass as bass  # noqa: E402
import concourse.tile as tile  # noqa: E402
from concourse import bacc, mybir  # noqa: E402
from concourse.bass_utils import run_bass_kernel_spmd  # noqa: E402

B, C, H, W = 32, 256, 64, 64
EPS = 1e-8
NCORES = 8
BL = B // NCORES  # batches per core: 4
HWX = H * W  # 4096
ROWS = BL * C  # 1024
NPB = HWX // 128  # position blocks per batch: 32
NCHUNK = C // 128  # channel chunks: 2
SOUT = 3 * NPB  # out cols per batch: 96

F32 = mybir.dt.float32
BF16 = mybir.dt.bfloat16

_CACHED_NC = None


def build_nc():
    nc = bacc.Bacc(
        "TRN2", target_bir_lowering=False, debug=False, num_devices=NCORES
    )
    u_d = nc.dram_tensor("u", [ROWS, HWX], F32, kind="ExternalInput")
    m_d = nc.dram_tensor("m", [ROWS, HWX], F32, kind="ExternalInput")
    ones_d = nc.dram_tensor("ones", [128, 1], BF16, kind="ExternalInput")
    # out col b*2*SOUT + ch*SOUT + s*NPB + pb, partition q: stat s of
    # channel-chunk ch at position pb*128+q (chunks summed on host)
    out_d = nc.dram_tensor(
        "out", [128, BL * NCHUNK * SOUT], F32, kind="ExternalOutput"
    )

    with tile.TileContext(nc) as tc, ExitStack() as ctx:
        const_pool = ctx.enter_context(tc.tile_pool(name="const", bufs=1))
        in_pool = ctx.enter_context(tc.tile_pool(name="inp", bufs=5))
        tmp_pool = ctx.enter_context(tc.tile_pool(name="tmp", bufs=3))
        st_pool = ctx.enter_context(tc.tile_pool(name="st", bufs=2))
        psum_pool = ctx.enter_context(
            tc.tile_pool(name="psum", bufs=4, space="PSUM")
        )

        ones_t = const_pool.tile([128, 1], BF16)
        nc.gpsimd.dma_start(ones_t[:], ones_d[:, :])

        # piece lists per (b, ch): column extents within the 4096 free dim.
        # Halves at the very start (fast ramp) and very end (short tail);
        # full 2 MiB DMAs otherwise (16 KiB descriptors hit SDMA line rate).
        def pieces(b, ch):
            first = b == 0 and ch == 0
            last = b == BL - 1 and ch == NCHUNK - 1
            if first or last:
                return [(0, 2048), (2048, 2048)]
            return [(0, HWX)]

        mm_ctr = 0
        for b in range(BL):
            # col ch*SOUT + s*NPB + pb (chunks side by side; added on host)
            P = psum_pool.tile([128, NCHUNK * SOUT], F32)
            st_t = st_pool.tile([128, NCHUNK * SOUT], F32, tag="st")
            for ch in range(NCHUNK):
                row0 = b * C + ch * 128
                for c0, ln in pieces(b, ch):
                    csl = slice(c0, c0 + ln)
                    u_t = in_pool.tile([128, ln], F32, tag="u")
                    nc.sync.dma_start(u_t[:], u_d[row0 : row0 + 128, csl])
                    # m rides the ACT HWDGE ring: separate issue path +
                    # slot-wait chain, so u slot stalls never block the m
                    # stream (and vice versa). SWDGE (gpsimd) is ~25% slower
                    # per byte — keep both big streams on HWDGE.
                    m_t = in_pool.tile([128, ln], F32, tag="m")
                    nc.scalar.dma_start(m_t[:], m_d[row0 : row0 + 128, csl])

                    # compute in half-tiles: smaller tmp slots recycle
                    # faster, decoupling DMA slot reuse from compute jitter
                    for h0 in range(0, ln, 2048):
                        hl = min(2048, ln - h0)
                        usl = u_t[:, h0 : h0 + hl]
                        msl = m_t[:, h0 : h0 + hl]
                        num_t = tmp_pool.tile([128, hl], BF16, tag="num")
                        nc.vector.tensor_mul(num_t[:], usl, msl)
                        uu_t = tmp_pool.tile([128, hl], BF16, tag="uu")
                        nc.scalar.square(uu_t[:], usl)
                        mm_t = tmp_pool.tile([128, hl], BF16, tag="mm")
                        # m*m: 1/3 DVE, 2/3 ACT keeps both engines under
                        # the DMA roofline (num must be DVE; ACT is the
                        # faster squarer)
                        if mm_ctr % 3 == 0:
                            nc.vector.tensor_mul(mm_t[:], msl, msl)
                        else:
                            nc.scalar.square(mm_t[:], msl)
                        mm_ctr += 1

                        pb0 = (c0 + h0) // 128
                        for s, t in enumerate((num_t, uu_t, mm_t)):
                            for j in range(hl // 128):
                                col = ch * SOUT + s * NPB + pb0 + j
                                nc.tensor.matmul(
                                    P[:, col : col + 1],
                                    t[:, j * 128 : (j + 1) * 128],
                                    ones_t[:, :],
                                    start=True,
                                    stop=True,
                                )

                # drain this chunk's stats while the other chunk streams
                psl = P[:, ch * SOUT : (ch + 1) * SOUT]
                ssl = st_t[:, ch * SOUT : (ch + 1) * SOUT]
                if ch == 0:
                    nc.scalar.copy(ssl, psl)
                else:
                    nc.vector.tensor_copy(ssl, psl)

            nc.gpsimd.dma_start(
                out_d[:, b * NCHUNK * SOUT : (b + 1) * NCHUNK * SOUT],
                st_t[:],
            )

    nc.compile()
    return nc


def get_nc():
    global _CACHED_NC
    if _CACHED_NC is None:
        _CACHED_NC = build_nc()
    return _CACHED_NC


def make_in_maps(unmasked, masked):
    ones = np.ones((128, 1), dtype=ml_dtypes.bfloat16)
    in_maps = []
    for i in range(NCORES):
        sl = slice(i * BL, (i + 1) * BL)
        u = np.ascontiguousarray(unmasked[sl]).reshape(ROWS, HWX)
        m = np.ascontiguousarray(masked[sl]).reshape(ROWS, HWX)
        in_maps.append({"u": u, "m": m, "ones": ones})
    return in_maps


def _finalize(results, latent_mask):
    num = 0.0
    den = 0.0
    for i, res in enumerate(results):
        out = np.asarray(res["out"], dtype=np.float64)
        out = out.reshape(128, BL, NCHUNK, 3, NPB).sum(axis=2)  # add chunks
        for b in range(BL):
            blk = out[:, b]  # [128, 3, NPB]
            # position p = pb*128 + partition -> transpose to [pb, part]
            num_p = blk[:, 0, :].T.reshape(-1)
            uu_p = blk[:, 1, :].T.reshape(-1)
            mm_p = blk[:, 2, :].T.reshape(-1)
            den_p = np.maximum(np.sqrt(np.maximum(uu_p, 0.0)), EPS) * \
                np.maximum(np.sqrt(np.maximum(mm_p, 0.0)), EPS)
            sim = num_p / den_p
            mask = latent_mask[i * BL + b].reshape(-1) != 0
            num += sim[mask].sum()
            den += float(mask.sum())
    return np.float32(num / den)


def kernel(unmasked_latent_tensors, masked_latent_tensors, latent_mask, **kw):
    nc = get_nc()
    in_maps = make_in_maps(
        np.asarray(unmasked_latent_tensors, dtype=np.float32),
        np.asarray(masked_latent_tensors, dtype=np.float32),
    )
    res = run_bass_kernel_spmd(nc, in_maps, list(range(NCORES)))
    return _finalize(res.results, np.asarray(latent_mask))


def kernel_traced(unmasked_latent_tensors, masked_latent_tensors, latent_mask):
    """Like kernel() but with NTFF tracing; returns (value, BassKernelResults)."""
    nc = get_nc()
    in_maps = make_in_maps(
        np.asarray(unmasked_latent_tensors, dtype=np.float32),
        np.asarray(masked_latent_tensors, dtype=np.float32),
    )
    res = run_bass_kernel_spmd(nc, in_maps, list(range(NCORES)), trace=True)
    return _finalize(res.results, np.asarray(latent_mask)), res


# revision 11
# speedup vs baseline: 1.1140x; 1.1140x over previous
"""Trainium2 Bass kernel for ContrastiveMaskedPatchSimilarity loss.

Computes: per-position cosine similarity along the channel axis of two
[32, 256, 64, 64] f32 tensors, then a masked mean -> scalar.

Strategy (pure data parallel over 8 NeuronCores, batch-sharded 4 each):
  - Layout on chip: [channel-chunk (128) = partitions, spatial (4096) = free].
    DMA of u/m tiles is contiguous per partition (16KB rows); mid-stream
    tiles are the full 2 MiB row block, with quarter tiles at the very
    start (fast ramp) and very end (short tail).
  - Elementwise products (u*m, u*u, m*m) on DVE/ACT, written as bf16.
  - Channel reduction via TensorE: product slice [128ch x 128pos] is the
    *stationary* operand (lhsT), rhs = ones[128,1] bf16 -> out[128pos, 1]
    lands position-major in PSUM. The two channel chunks accumulate into
    the same PSUM column (start=ch==0, stop=ch==1).
  - Per batch: one [128, 3*NPB] PSUM->SBUF copy, DMA'd straight to DRAM.
    The cosine division, mask, and final mean run on the host (tiny).
"""

import sys
from contextlib import ExitStack

import numpy as np

sys.path.insert(0, "/opt/trn_rl_repo")

import ml_dtypes  # noqa: E402

import concourse.bass as bass  # noqa: E402
import concourse.tile as tile  # noqa: E402
from concourse import bacc, mybir  # noqa: E402
from concourse.bass_utils import run_bass_kernel_spmd  # noqa: E402

B, C, H, W = 32, 256, 64, 64
EPS = 1e-8
NCORES = 8
BL = B // NCORES  # batches per core: 4
HWX = H * W  # 4096
ROWS = BL * C  # 1024
NPB = HWX // 128  # position blocks per batch: 32
NCHUNK = C // 128  # channel chunks: 2
SOUT = 3 * NPB  # out cols per batch: 96

F32 = mybir.dt.float32
BF16 = mybir.dt.bfloat16

_CACHED_NC = None


def build_nc():
    nc = bacc.Bacc(
        "TRN2", target_bir_lowering=False, debug=False, num_devices=NCORES
    )
    u_d = nc.dram_tensor("u", [ROWS, HWX], F32, kind="ExternalInput")
    m_d = nc.dram_tensor("m", [ROWS, HWX], F32, kind="ExternalInput")
    ones_d = nc.dram_tensor("ones", [128, 1], BF16, kind="ExternalInput")
    # out col b*2*SOUT + ch*SOUT + s*NPB + pb, partition q: stat s of
    # channel-chunk ch at position pb*128+q (chunks summed on host)
    out_d = nc.dram_tensor(
        "out", [128, BL * NCHUNK * SOUT], F32, kind="ExternalOutput"
    )

    with tile.TileContext(nc) as tc, ExitStack() as ctx:
        const_pool = ctx.enter_context(tc.tile_pool(name="const", bufs=1))
        in_pool = ctx.enter_context(tc.tile_pool(name="inp", bufs=5))
        tmp_pool = ctx.enter_context(tc.tile_pool(name="tmp", bufs=3))
        st_pool = ctx.enter_context(tc.tile_pool(name="st", bufs=2))
        psum_pool = ctx.enter_context(
            tc.tile_pool(name="psum", bufs=4, space="PSUM")
        )

        ones_t = const_pool.tile([128, 1], BF16)
        nc.gpsimd.dma_start(ones_t[:], ones_d[:, :])

        # piece lists per (b, ch): column extents within the 4096 free dim.
        # Halves at the very start (fast ramp) and very end (short tail);
        # full 2 MiB DMAs otherwise (16 KiB descriptors hit SDMA line rate).
        def pieces(b, ch):
            first = b == 0 and ch == 0
            last = b == BL - 1 and ch == NCHUNK - 1
            if first or last:
                return [(0, 2048), (2048, 2048)]
            return [(0, HWX)]

        mm_ctr = 0
        for b in range(BL):
            # col ch*SOUT + s*NPB + pb (chunks side by side; added on host)
            P = psum_pool.tile([128, NCHUNK * SOUT], F32)
            st_t = st_pool.tile([128, NCHUNK * SOUT], F32, tag="st")
            for ch in range(NCHUNK):
                row0 = b * C + ch * 128
                for c0, ln in pieces(b, ch):
                    csl = slice(c0, c0 + ln)
                    u_t = in_pool.tile([128, ln], F32, tag="u")
                    nc.sync.dma_start(u_t[:], u_d[row0 : row0 + 128, csl])
                    # Both streams ride the SP HWDGE ring: SWDGE (gpsimd)
                    # is ~25% slower per byte, and the ACT ring's issues
                    # get stuck behind ACT compute in program order.
                    m_t = in_pool.tile([128, ln], F32, tag="m")
                    nc.sync.dma_start(m_t[:], m_d[row0 : row0 + 128, csl])

                    # compute in half-tiles: smaller tmp slots recycle
                    # faster, decoupling DMA slot reuse from compute jitter
                    for h0 in range(0, ln, 2048):
                        hl = min(2048, ln - h0)
                        usl = u_t[:, h0 : h0 + hl]
                        msl = m_t[:, h0 : h0 + hl]
                        num_t = tmp_pool.tile([128, hl], BF16, tag="num")
                        nc.vector.tensor_mul(num_t[:], usl, msl)
                        uu_t = tmp_pool.tile([128, hl], BF16, tag="uu")
                        nc.scalar.square(uu_t[:], usl)
                        mm_t = tmp_pool.tile([128, hl], BF16, tag="mm")
                        # m*m: 1/3 DVE, 2/3 ACT keeps both engines under
                        # the DMA roofline (num must be DVE; ACT is the
                        # faster squarer)
                        if mm_ctr % 3 == 0:
                            nc.vector.tensor_mul(mm_t[:], msl, msl)
                        else:
                            nc.scalar.square(mm_t[:], msl)
                        mm_ctr += 1

                        pb0 = (c0 + h0) // 128
                        for s, t in enumerate((num_t, uu_t, mm_t)):
                            for j in range(hl // 128):
                                col = ch * SOUT + s * NPB + pb0 + j
                                nc.tensor.matmul(
                                    P[:, col : col + 1],
                                    t[:, j * 128 : (j + 1) * 128],
                                    ones_t[:, :],
                                    start=True,
                                    stop=True,
                                )

                # drain this chunk's stats while the other chunk streams
                psl = P[:, ch * SOUT : (ch + 1) * SOUT]
                ssl = st_t[:, ch * SOUT : (ch + 1) * SOUT]
                if ch == 0:
                    nc.scalar.copy(ssl, psl)
                else:
                    nc.vector.tensor_copy(ssl, psl)

            nc.gpsimd.dma_start(
                out_d[:, b * NCHUNK * SOUT : (b + 1) * NCHUNK * SOUT],
                st_t[:],
            )

    nc.compile()
    return nc


def get_nc():
    global _CACHED_NC
    if _CACHED_NC is None:
        _CACHED_NC = build_nc()
    return _CACHED_NC


def make_in_maps(unmasked, masked):
    ones = np.ones((128, 1), dtype=ml_dtypes.bfloat16)
    in_maps = []
    for i in range(NCORES):
        sl = slice(i * BL, (i + 1) * BL)
        u = np.ascontiguousarray(unmasked[sl]).reshape(ROWS, HWX)
        m = np.ascontiguousarray(masked[sl]).reshape(ROWS, HWX)
        in_maps.append({"u": u, "m": m, "ones": ones})
    return in_maps


def _finalize(results, latent_mask):
    num = 0.0
    den = 0.0
    for i, res in enumerate(results):
        out = np.asarray(res["out"], dtype=np.float64)
        out = out.reshape(128, BL, NCHUNK, 3, NPB).sum(axis=2)  # add chunks
        for b in range(BL):
            blk = out[:, b]  # [128, 3, NPB]
            # position p = pb*128 + partition -> transpose to [pb, part]
            num_p = blk[:, 0, :].T.reshape(-1)
            uu_p = blk[:, 1, :].T.reshape(-1)
            mm_p = blk[:, 2, :].T.reshape(-1)
            den_p = np.maximum(np.sqrt(np.maximum(uu_p, 0.0)), EPS) * \
                np.maximum(np.sqrt(np.maximum(mm_p, 0.0)), EPS)
            sim = num_p / den_p
            mask = latent_mask[i * BL + b].reshape(-1) != 0
            num += sim[mask].sum()
            den += float(mask.sum())
    return np.float32(num / den)


def kernel(unmasked_latent_tensors, masked_latent_tensors, latent_mask, **kw):
    nc = get_nc()
    in_maps = make_in_maps(
        np.asarray(unmasked_latent_tensors, dtype=np.float32),
        np.asarray(masked_latent_tensors, dtype=np.float32),
    )
    res = run_bass_kernel_spmd(nc, in_maps, list(range(NCORES)))
    return _finalize(res.results, np.asarray(latent_mask))


def kernel_traced(unmasked_latent_tensors, masked_latent_tensors, latent_mask):
    """Like kernel() but with NTFF tracing; returns (value, BassKernelResults)."""
    nc = get_nc()
    in_maps = make_in_maps(
        np.asarray(unmasked_latent_tensors, dtype=np.float32),
        np.asarray(masked_latent_tensors, dtype=np.float32),
    )
    res = run_bass_kernel_spmd(nc, in_maps, list(range(NCORES)), trace=True)
    return _finalize(res.results, np.asarray(latent_mask)), res
